# revision 1
# baseline (speedup 1.0000x reference)
"""L2-distance attention (B=4, DIM=512, N=2048, H=8, D=32) on 8 trn2 NeuronCores.

Sharding: core c handles batch b = c//2, query-half = c%2 (1024 queries, all
2048 keys, all 8 heads).  Output is a pure concat — no cross-core reduce.

Per-core pipeline (all on one NeuronCore, Tile-scheduled):
  A. q = w_q^T x (half), k = w_k^T x (full), v^T = x^T w_v (j-major, with a
     ones column per head folded in for the softmax row-sums).
  B. per head: dist2 computed directly by PE via augmented vectors
     k~=[k; k2; 1], q~=[-2q; 1; q2]  ->  k~.q~ = ||q-k||^2 in PSUM.
     ACT: sqrt (bias=delta guard), then exp(-scale * s) -> E (unnormalized
     softmax numerator; logits are always <= 0 so no max-subtraction).
     PE: out_un = [V^T; 1]^T E  -> rows 0..31 = attn@v (unnormalized),
     row 32 = row-sums.  DVE reciprocal + PE outer-product broadcast +
     DVE multiply -> normalized head output, DMA'd into Y (head-major).
  C. Z = w_out^T Y + b, DMA to DRAM.
"""

import numpy as np

import concourse.bass as bass
import concourse.mybir as mybir
import concourse.tile as tile
from concourse import bacc

F32 = mybir.dt.float32
F32R = mybir.dt.float32r
AF = mybir.ActivationFunctionType


def R(ap):
    return ap.bitcast(F32R)


def make_mm(nc):
    def mm(out, lhsT, rhs, start, stop):
        nc.tensor.matmul(out, R(lhsT), R(rhs), start=start, stop=stop)
    return mm


B, DIM, N = 4, 512, 2048
H, D = 8, 32
INNER = H * D            # 256
NQ = N // 2              # 1024 queries per core
P = 128
KT = DIM // P            # 4 contraction tiles for the projections
NJT = N // P             # 16 key tiles
VTW = D + 1              # 33: v columns + ones column per head
VSTRIDE = H * VTW        # 264 columns per key-tile block of vt
SCALE = float(D) ** -0.5
DELTA = 0.02             # sqrt-domain guard against f32r rounding of dist2
NEQ = 4                  # E quarters (each covers NJT//NEQ key tiles)
JQ = NJT // NEQ          # 4 key tiles per E quarter
KA = 65                  # augmented contraction: [32 | flag@32 | zeros | flag@64]


def build_program() -> bass.Bass:
    nc = bacc.Bacc("TRN2", target_bir_lowering=False, debug=False)

    xq_d = nc.declare_dram_parameter("xq", [DIM, NQ], F32, isOutput=False)
    xkv_d = nc.declare_dram_parameter("xkv", [DIM, N], F32, isOutput=False)
    wq_d = nc.declare_dram_parameter("wq", [DIM, INNER], F32, isOutput=False)
    wkv_d = nc.declare_dram_parameter("wkv", [DIM, 2 * INNER], F32, isOutput=False)
    wo_d = nc.declare_dram_parameter("wo", [INNER, DIM], F32, isOutput=False)
    b_d = nc.declare_dram_parameter("b", [DIM], F32, isOutput=False)
    z_d = nc.declare_dram_parameter("z", [DIM, NQ], F32, isOutput=True)

    with tile.TileContext(nc) as tc:
        mm = make_mm(nc)
        with tc.tile_pool(name="keep", bufs=1) as keep, \
             tc.tile_pool(name="work", bufs=2) as work:

            # ---- persistent tiles ----
            q_t = [keep.tile([P, NQ], F32, tag=f"q{m}", name=f"q{m}") for m in range(2)]
            k_t = [keep.tile([P, N], F32, tag=f"k{m}", name=f"k{m}") for m in range(2)]
            vt_big = keep.tile([P, NJT * VSTRIDE], F32, tag="vt", name="vt")
            y_t = [keep.tile([P, NQ], F32, tag=f"y{m}", name=f"y{m}") for m in range(2)]
            wo_t = [keep.tile([P, DIM], F32, tag=f"wo{m}", name=f"wo{m}") for m in range(2)]
            b_t = keep.tile([P, KT], F32, tag="bias", name="bias")
            ones = keep.tile([64, 32], F32, tag="ones", name="ones")
            delta_t = keep.tile([P, 1], F32, tag="delta", name="delta")
            zero_t = keep.tile([P, 1], F32, tag="zero", name="zero")
            onesP = keep.tile([P, 1], F32, tag="onesP", name="onesP")

            # memset cannot write f32r; set plain f32 constants and
            # broadcast-copy (DVE copy CAN round to f32r) where needed.
            nc.vector.memset(onesP[:, :], 1.0)
            nc.vector.memset(delta_t[:, :], DELTA)
            nc.vector.memset(zero_t[:, :], 0.0)
            nc.vector.tensor_copy(R(ones[:, :]),
                                  onesP[0:64, 0:1].to_broadcast((64, 32)))
            wo_r = wo_d[:].rearrange("(t p) o -> t p o", p=P)
            for m in range(2):
                nc.sync.dma_start(out=R(wo_t[m][:, :]), in_=R(wo_r[m]))
            nc.sync.dma_start(out=b_t[:, :], in_=b_d[:].rearrange("(t p) -> p t", p=P))

            # ======== Phase A: projections ========
            with tc.tile_pool(name="xw", bufs=1) as xw, \
                 tc.tile_pool(name="pp", bufs=3, space="PSUM") as pp:
                wq_t = [xw.tile([P, INNER], F32, tag=f"wq{k}", name=f"wq{k}") for k in range(KT)]
                wkv_t = [xw.tile([P, 2 * INNER], F32, tag=f"wkv{k}", name=f"wkv{k}") for k in range(KT)]
                xq_t = [xw.tile([P, NQ], F32, tag=f"xq{k}", name=f"xq{k}") for k in range(KT)]
                xkv_t = [xw.tile([P, N], F32, tag=f"xkv{k}", name=f"xkv{k}") for k in range(KT)]

                xq_r = xq_d[:].rearrange("(t p) n -> t p n", p=P)
                xkv_r = xkv_d[:].rearrange("(t p) n -> t p n", p=P)
                wq_r = wq_d[:].rearrange("(t p) o -> t p o", p=P)
                wkv_r = wkv_d[:].rearrange("(t p) o -> t p o", p=P)
                for k in range(KT):
                    nc.sync.dma_start(out=R(xq_t[k][:, :]), in_=R(xq_r[k]))
                    nc.sync.dma_start(out=R(xkv_t[k][:, :]), in_=R(xkv_r[k]))
                    nc.sync.dma_start(out=R(wq_t[k][:, :]), in_=R(wq_r[k]))
                    nc.sync.dma_start(out=R(wkv_t[k][:, :]), in_=R(wkv_r[k]))

                # q projection: (DIM x NQ) -> (INNER x NQ)
                for m in range(2):
                    for n in range(NQ // 512):
                        ps = pp.tile([P, 512], F32, tag="proj", name="proj")
                        for k in range(KT):
                            mm(ps[:, :],
                               wq_t[k][:, m * P:(m + 1) * P],
                               xq_t[k][:, n * 512:(n + 1) * 512],
                               start=(k == 0), stop=(k == KT - 1))
                        nc.vector.tensor_copy(R(q_t[m][:, n * 512:(n + 1) * 512]), ps[:, :])

                # k projection: (DIM x N) -> (INNER x N)   (wkv cols 0:256)
                for m in range(2):
                    for n in range(N // 512):
                        ps = pp.tile([P, 512], F32, tag="proj", name="proj")
                        for k in range(KT):
                            mm(ps[:, :],
                               wkv_t[k][:, m * P:(m + 1) * P],
                               xkv_t[k][:, n * 512:(n + 1) * 512],
                               start=(k == 0), stop=(k == KT - 1))
                        nc.vector.tensor_copy(R(k_t[m][:, n * 512:(n + 1) * 512]), ps[:, :])

                # v^T projection: per key tile jt, (128 j x 256 d), strided into
                # vt_big so each head's 32 columns sit next to its ones column.
                nc.vector.tensor_copy(
                    R(vt_big[:, :].rearrange("p (a c) -> p a c", c=VTW)[:, :, D:D + 1]),
                    onesP[:, 0:1].to_broadcast((P, P, 1)))
                for jt in range(NJT):
                    ps = pp.tile([P, INNER], F32, tag="vtps", name="vtps")
                    for k in range(KT):
                        mm(ps[:, :],
                           xkv_t[k][:, jt * P:(jt + 1) * P],
                           wkv_t[k][:, INNER:2 * INNER],
                           start=(k == 0), stop=(k == KT - 1))
                    dst = vt_big[:, jt * VSTRIDE:(jt + 1) * VSTRIDE] \
                        .rearrange("p (h c) -> p h c", c=VTW)[:, :, 0:D]
                    src = ps[:, :].rearrange("p (h d) -> p h d", d=D)
                    nc.vector.tensor_copy(R(dst), src)

            # ======== Phase B: attention heads ========
            # Augmented-vector tiles are persistent ping-pong pairs so the
            # constant rows (ones/zeros padding) are written once, not per
            # head.  The normalization tail of head h-1 is emitted inside
            # head h so its PE ops queue AFTER head h's dist2 matmuls —
            # keeps the in-order PE stream free of the reciprocal stall.
            with tc.tile_pool(name="epool", bufs=NEQ, space="SBUF") as epool, \
                 tc.tile_pool(name="pd2", bufs=2, space="PSUM") as pd2, \
                 tc.tile_pool(name="po", bufs=1, space="PSUM") as po, \
                 tc.tile_pool(name="pm", bufs=2, space="PSUM") as pm:
                kt_t = [keep.tile([KA, N], F32, tag=f"kt{i}", name=f"kt{i}")
                        for i in range(2)]
                qt_t = [keep.tile([KA, NQ], F32, tag=f"qt{i}", name=f"qt{i}")
                        for i in range(2)]
                for i in range(2):
                    nc.vector.tensor_copy(R(kt_t[i][D:2 * D, :]),
                                          zero_t[0:D, 0:1].to_broadcast((D, N)))
                    nc.vector.tensor_copy(R(kt_t[i][D:D + 1, :]),
                                          onesP[0:1, 0:1].to_broadcast((1, N)))
                    nc.vector.tensor_copy(R(qt_t[i][D:2 * D, :]),
                                          zero_t[0:D, 0:1].to_broadcast((D, NQ)))
                    nc.vector.tensor_copy(R(qt_t[i][2 * D:2 * D + 1, :]),
                                          onesP[0:1, 0:1].to_broadcast((1, NQ)))

                po_s = [work.tile([VTW, NQ], F32, tag=f"pos{i}", name=f"pos{i}",
                                  bufs=1) for i in range(2)]
                tail = {}

                def emit_tail(ph):
                    # deferred normalization of head ph.  The (1,1024) row of
                    # row-sums is reciprocal'd as (128,8) — a single-partition
                    # reciprocal costs ~6.4ns/element, partition-parallel is
                    # ~100x faster — via a scatter DMA there and back.
                    pmt, pmo, psrc = tail.pop(ph)
                    rs128 = work.tile([P, NQ // P], F32, tag="rs", name="rs")
                    nc.sync.dma_start(out=rs128[:, :], in_=psrc[D:D + 1, :])
                    rr128 = work.tile([P, NQ // P], F32, tag="rr", name="rr")
                    with nc.allow_low_precision(reason="f32r full fp32 range"):
                        nc.vector.reciprocal(R(rr128[:, :]), rs128[:, :])
                    rrow = work.tile([1, NQ], F32, tag="rrow", name="rrow")
                    nc.sync.dma_start(out=R(rrow[:, :]), in_=R(rr128[:, :]))
                    prep = pd2.tile([D, NQ], F32, tag="d2", name="d2")
                    for n in range(NQ // 512):
                        mm(prep[:, n * 512:(n + 1) * 512],
                           ones[0:1, 0:D],
                           rrow[:, n * 512:(n + 1) * 512],
                           start=True, stop=True)
                    nc.vector.tensor_mul(R(y_t[pmt][pmo:pmo + D, :]),
                                         psrc[0:D, :], prep[:, :])

                from contextlib import nullcontext

                for h in range(H):
                    mt, mo = h // 4, (h % 4) * D
                    q_h = q_t[mt][mo:mo + D, :]
                    k_h = k_t[mt][mo:mo + D, :]
                    kt = kt_t[h % 2]
                    qt = qt_t[h % 2]
                    prio = tc.high_priority(10000) if h == 0 else nullcontext()
                    prio.__enter__()

                    # --- per-head rows of k~/q~ (all DVE) ---
                    nc.vector.tensor_scalar_mul(R(kt[0:D, :]), k_h, -2.0)
                    ksq = work.tile([D, N], F32, tag="ksq", name="ksq", bufs=1)
                    nc.vector.tensor_mul(R(ksq[:, :]), k_h, k_h)
                    for n in range(N // 512):
                        k2ps = pm.tile([1, 512], F32, tag="misc", name="misc")
                        mm(k2ps[:, :], ones[0:D, 0:1],
                           ksq[:, n * 512:(n + 1) * 512], start=True, stop=True)
                        nc.vector.tensor_copy(
                            R(kt[2 * D:2 * D + 1, n * 512:(n + 1) * 512]), k2ps[:, :])
                    nc.vector.tensor_copy(R(qt[0:D, :]), q_h)
                    qsq = work.tile([D, NQ], F32, tag="qsq", name="qsq", bufs=1)
                    nc.vector.tensor_mul(R(qsq[:, :]), q_h, q_h)
                    for n in range(NQ // 512):
                        q2ps = pm.tile([1, 512], F32, tag="misc", name="misc")
                        mm(q2ps[:, :], ones[0:D, 0:1],
                           qsq[:, n * 512:(n + 1) * 512], start=True, stop=True)
                        nc.vector.tensor_copy(
                            R(qt[D:D + 1, n * 512:(n + 1) * 512]), q2ps[:, :])

                    # --- dist2 (PE) -> sqrt -> exp (ACT) ---
                    eq = [epool.tile([P, JQ * NQ], F32, tag="eq", name="eq")
                          for _ in range(NEQ)]
                    for jt in range(NJT):
                        psd = pd2.tile([P, NQ], F32, tag="d2", name="d2")
                        for n in range(NQ // 512):
                            mm(psd[:, n * 512:(n + 1) * 512],
                               kt[:, jt * P:(jt + 1) * P],
                               qt[:, n * 512:(n + 1) * 512],
                               start=True, stop=True)
                        nc.scalar.activation(
                            R(eq[jt // JQ][:, (jt % JQ) * NQ:(jt % JQ + 1) * NQ]),
                            psd[:, :], AF.Sqrt, bias=delta_t[:, :], scale=1.0)

                    prio.__exit__(None, None, None)

                    # head h-1 tail: PE ops queue here, after dist2(h)
                    if h - 1 in tail:
                        emit_tail(h - 1)

                    for qq in range(NEQ):
                        nc.scalar.activation(R(eq[qq][:, :]), eq[qq][:, :],
                                             AF.Exp, bias=zero_t[:, :], scale=-SCALE)

                    # --- attn @ v with fused row-sums ---
                    pso = po.tile([VTW, NQ], F32, tag="o", name="o")
                    for jt in range(NJT):
                        ebase = (jt % JQ) * NQ
                        for n in range(NQ // 512):
                            mm(pso[:, n * 512:(n + 1) * 512],
                               vt_big[:, jt * VSTRIDE + h * VTW:
                                      jt * VSTRIDE + (h + 1) * VTW],
                               eq[jt // JQ][:, ebase + n * 512:ebase + (n + 1) * 512],
                               start=(jt == 0), stop=(jt == NJT - 1))
                    # move to SBUF so the PSUM slot frees immediately
                    psrc = po_s[h % 2]
                    nc.vector.tensor_copy(R(psrc[:, :]), pso[:, :])
                    tail[h] = (mt, mo, psrc)

                # last head's tail
                emit_tail(H - 1)

            # ======== Phase C: output projection + bias ========
            with tc.tile_pool(name="pz", bufs=2, space="PSUM") as pz:
                z_r = z_d[:].rearrange("(t p) n -> t p n", p=P)
                for m in range(KT):
                    ps = pz.tile([P, NQ], F32, tag="z", name="z")
                    for n in range(NQ // 512):
                        for k in range(2):
                            mm(ps[:, n * 512:(n + 1) * 512],
                               wo_t[k][:, m * P:(m + 1) * P],
                               y_t[k][:, n * 512:(n + 1) * 512],
                               start=(k == 0), stop=(k == 1))
                    zt = work.tile([P, NQ], F32, tag="ytmp", name="ytmp")
                    nc.vector.tensor_scalar_add(zt[:, :], ps[:, :], b_t[:, m:m + 1])
                    nc.sync.dma_start(out=z_r[m], in_=zt[:, :])

    nc.compile()
    return nc


def make_in_maps(x, w_qkv, w_out, b_out):
    x = np.asarray(x, dtype=np.float32)
    w_qkv = np.asarray(w_qkv, dtype=np.float32)
    w_out = np.asarray(w_out, dtype=np.float32)
    b_out = np.asarray(b_out, dtype=np.float32)
    w_qT = np.ascontiguousarray(w_qkv[0:INNER, :].T)          # (DIM, INNER)
    w_kvT = np.ascontiguousarray(w_qkv[INNER:3 * INNER, :].T)  # (DIM, 512)
    w_oT = np.ascontiguousarray(w_out.T)                       # (INNER, DIM)
    in_maps = []
    for c in range(8):
        b, half = c // 2, c % 2
        in_maps.append({
            "xq": np.ascontiguousarray(x[b][:, half * NQ:(half + 1) * NQ]),
            "xkv": np.ascontiguousarray(x[b]),
            "wq": w_qT,
            "wkv": w_kvT,
            "wo": w_oT,
            "b": b_out,
        })
    return in_maps


def assemble_output(results):
    out = np.empty((B, DIM, N), dtype=np.float32)
    for c in range(8):
        b, half = c // 2, c % 2
        out[b][:, half * NQ:(half + 1) * NQ] = results[c]["z"]
    return out


_prog_cache = {}


def kernel(x, w_qkv, w_out, b_out):
    from concourse.bass_utils import run_bass_kernel_spmd
    if "nc" not in _prog_cache:
        _prog_cache["nc"] = build_program()
    nc = _prog_cache["nc"]
    in_maps = make_in_maps(x, w_qkv, w_out, b_out)
    res = run_bass_kernel_spmd(nc, in_maps, list(range(8)))
    return assemble_output(res.results)

